# revision 4
# baseline (speedup 1.0000x reference)
"""DIGNN-RW fixed-point GNN on 8 Trainium2 NeuronCores.

Strategy (node-sharded, z replicated):
- 100000 nodes -> 8 cores x 12500, padded to 12544 = 98 windows x 128 rows.
- z state [100352, 128] bf16 replicated in each core's DRAM, refreshed per
  fixed-point iteration by an 8-core AllGather (~3.2MB/rank).
- Per row-window: dma_gather pulls all window edges' z[col] rows (bf16, 256B
  rows; int16 indices force 4 column-groups of 25088 rows); per 128-edge chunk
  a one-hot matrix S[edge, row] * c_e is built on the vector engine
  (tensor_scalar is_equal*mult against an iota tile) and TensorE accumulates
  S.T @ Zg into a PSUM row-window; epilogue adds 0.5*h.
- Edge coefficients c_e = w_e * deg_inv[row] / (1+mu) folded on host.
- Encoder MLP+BN, node FC stack, graph pooling (one-hot batch matmuls +
  AllReduce), graph FC stack and log_softmax all run on device; every core
  computes the (tiny) graph stage redundantly and core 0's output is returned.
"""
import sys
import numpy as np

sys.path.insert(0, "/opt/trn_rl_repo")

from concourse import bass, mybir, bacc, tile  # noqa: E402
from concourse import bass_utils  # noqa: E402

# problem constants
N = 100_000
E = 1_600_000
G = 512
CIN = 128
H = 128
COUT = 10
MU = 1.0
BN_EPS = 1e-5
MAX_ITER = 10

NC = 8
P = 128
NPC = N // NC                 # 12500 real nodes per core
WPC = (NPC + P - 1) // P      # 98 windows per core
SH = WPC * P                  # 12544 padded rows per core
NPAD = NC * SH                # 100352
NG = 4                        # column groups (int16 gather index limit)
GSZ = NPAD // NG              # 25088 rows per group
NITER = MAX_ITER - 1          # SpMM applications (z1 = 0.5*h costs nothing)

_F32 = mybir.dt.float32
_BF16 = mybir.dt.bfloat16
_I16 = mybir.dt.int16


def _zrow(i):
    """Padded z-table row for original node i (partition-major shard layout)."""
    c = i // NPC
    j = i - c * NPC
    w = j // P
    p = j - w * P
    return c * SH + p * WPC + w


def _prep(inputs):
    x = np.asarray(inputs["x"], np.float32)
    ei = np.asarray(inputs["edge_index"], np.int64)
    ew = np.asarray(inputs["edge_weight"], np.float32)
    batch = np.asarray(inputs["batch"], np.int64)

    row, col = ei[0].astype(np.int64), ei[1].astype(np.int64)
    deg = np.bincount(row, weights=ew.astype(np.float64), minlength=N).astype(np.float32)
    deg_inv = 1.0 / np.clip(deg, 1e-12, None)
    cval = (ew * deg_inv[row] / (1.0 + MU)).astype(np.float32)

    # edge -> (core, window, colgroup)
    ecore = row // NPC
    j = row - ecore * NPC
    ew_win = j // P
    erow_p = (j - ew_win * P).astype(np.float32)     # rr in [0,128)
    zc = _zrow(col)
    eg = zc // GSZ
    erel = (zc - eg * GSZ).astype(np.int32)          # < 25088, int16-safe

    # bucket id and stable sort by (bucket, col) for HBM locality
    bucket = ((ecore * WPC + ew_win) * NG + eg).astype(np.int64)
    order = np.lexsort((erel, bucket))
    bucket_s = bucket[order]
    counts = np.bincount(bucket_s, minlength=NC * WPC * NG)
    starts = np.concatenate(([0], np.cumsum(counts)))[:-1]
    rank = np.arange(E, dtype=np.int64) - starts[bucket_s]

    cnt3 = counts.reshape(NC, WPC, NG)
    cpw_g = [max(1, int(np.ceil(cnt3[:, :, g].max() / P))) for g in range(NG)]
    cpt = sum(cpw_g)
    off_g = np.concatenate(([0], np.cumsum(cpw_g)))[:-1]

    # padded slot layout: per (c, w, g): slots cpw_g[g]*128; slot i=(k*128+p)
    rr_all = np.zeros((NC, P, WPC * cpt), np.float32)
    cc_all = np.zeros((NC, P, WPC * cpt), np.float32)

    bs = bucket_s
    cs = (bs // (WPC * NG))
    ws = (bs // NG) % WPC
    gs = bs % NG
    slot = rank                                       # within bucket
    k = slot // P
    pslot = slot % P
    colidx = ws * cpt + off_g[gs] + k
    rr_all[cs, pslot, colidx] = erow_p[order]
    cc_all[cs, pslot, colidx] = cval[order]
    # idx tensors per group: [NC, 128, WPC * cpw_g[g] * 8] int16 (16-row data x8)
    erel_s = erel[order].astype(np.int16)
    idx_pg = []
    for g in range(NG):
        K8 = cpw_g[g] * 8
        arr = np.zeros((NC, 16, WPC * K8), np.int16)
        mg = gs == g
        arr[cs[mg], slot[mg] % 16, ws[mg] * K8 + slot[mg] // 16] = erel_s[mg]
        idx_pg.append(np.tile(arr, (1, 8, 1)))

    # x shards + batch ids (dummy nodes excluded from pooling via id 600)
    i_all = np.arange(N, dtype=np.int64)
    c_all_n = i_all // NPC
    j_all = i_all - c_all_n * NPC
    w_all = j_all // P
    p_all = j_all - w_all * P
    x_sh = np.zeros((NC, P, WPC, CIN), np.float32)
    x_sh[c_all_n, p_all, w_all, :] = x
    batchf = np.full((NC, P, WPC), 600.0, np.float32)
    batchf[c_all_n, p_all, w_all] = batch.astype(np.float32)

    # weights (bf16 for matmul stationary operands, fp32 biases/scales)
    from ml_dtypes import bfloat16
    tobf = lambda a: np.asarray(a, np.float32).astype(bfloat16)
    s = np.asarray(inputs["bn_gamma"], np.float32) / np.sqrt(np.asarray(inputs["bn_var"], np.float32) + BN_EPS)
    hb_scale = (0.5 * s).astype(np.float32)[:, None]
    hb_bias = (0.5 * ((np.asarray(inputs["mlp_b3"], np.float32) - np.asarray(inputs["bn_mean"], np.float32)) * s
                      + np.asarray(inputs["bn_beta"], np.float32))).astype(np.float32)[:, None]

    iota128 = np.broadcast_to(np.arange(P, dtype=np.float32), (P, P)).copy()
    iota512 = np.broadcast_to(np.arange(G, dtype=np.float32), (P, G)).copy()
    ident = np.eye(P, dtype=np.float32)

    common = dict(
        w1=tobf(inputs["mlp_w1"]), b1=np.asarray(inputs["mlp_b1"], np.float32)[:, None],
        w2=tobf(inputs["mlp_w2"]), b2=np.asarray(inputs["mlp_b2"], np.float32)[:, None],
        w3=tobf(inputs["mlp_w3"]),
        hb_scale=hb_scale, hb_bias=hb_bias,
        fcw0=tobf(np.asarray(inputs["fc_w"])[0]), fcb0=np.asarray(inputs["fc_b"], np.float32)[0][:, None],
        fcw1=tobf(np.asarray(inputs["fc_w"])[1]), fcb1=np.asarray(inputs["fc_b"], np.float32)[1][:, None],
        gfcw0=tobf(np.asarray(inputs["gfc_w"])[0]), gfcb0=np.asarray(inputs["gfc_b"], np.float32)[0][:, None],
        gfcw1=tobf(np.asarray(inputs["gfc_w"])[1]), gfcb1=np.asarray(inputs["gfc_b"], np.float32)[1][:, None],
        finw=tobf(inputs["final_w"]), finb=np.asarray(inputs["final_b"], np.float32)[:, None],
        iota128=tobf(iota128), iota512=iota512,
        ident=tobf(ident), identf=ident,
    )

    in_maps = []
    for c in range(NC):
        m = dict(common)
        m["x_sh"] = x_sh[c]
        m["batchf"] = batchf[c]
        m["rr_all"] = rr_all[c]
        m["cc_all"] = cc_all[c]
        for g in range(NG):
            m[f"idx{g}"] = idx_pg[g][c]
        in_maps.append(m)
    return in_maps, cpw_g, cpt, off_g


def _build(cpw_g, cpt, off_g, niter=NITER):
    nc = bacc.Bacc("TRN2", target_bir_lowering=False, debug=False,
                   enable_asserts=False, num_devices=NC)
    AF = mybir.ActivationFunctionType
    OP = mybir.AluOpType

    # inputs
    x_sh = nc.dram_tensor("x_sh", [P, WPC, CIN], _F32, kind="ExternalInput")
    batchf = nc.dram_tensor("batchf", [P, WPC], _F32, kind="ExternalInput")
    rr_in = nc.dram_tensor("rr_all", [P, WPC * cpt], _F32, kind="ExternalInput")
    cc_in_t = nc.dram_tensor("cc_all", [P, WPC * cpt], _F32, kind="ExternalInput")
    idx_in = [nc.dram_tensor(f"idx{g}", [P, WPC * cpw_g[g] * 8], _I16, kind="ExternalInput")
              for g in range(NG)]
    wts = {}
    for nm, shp, dt in [
        ("w1", [CIN, 64], _BF16), ("b1", [64, 1], _F32),
        ("w2", [64, H], _BF16), ("b2", [H, 1], _F32),
        ("w3", [H, H], _BF16),
        ("hb_scale", [H, 1], _F32), ("hb_bias", [H, 1], _F32),
        ("fcw0", [H, H], _BF16), ("fcb0", [H, 1], _F32),
        ("fcw1", [H, H], _BF16), ("fcb1", [H, 1], _F32),
        ("gfcw0", [H, H], _BF16), ("gfcb0", [H, 1], _F32),
        ("gfcw1", [H, H], _BF16), ("gfcb1", [H, 1], _F32),
        ("finw", [H, COUT], _BF16), ("finb", [COUT, 1], _F32),
        ("iota128", [P, P], _BF16), ("iota512", [P, G], _F32),
        ("ident", [P, P], _BF16), ("identf", [P, P], _F32),
    ]:
        wts[nm] = nc.dram_tensor(nm, shp, dt, kind="ExternalInput")
    out = nc.dram_tensor("out", [G, COUT], _F32, kind="ExternalOutput")

    with tile.TileContext(nc) as tc:
        with tc.tile_pool(name="res", bufs=1) as res, \
             tc.tile_pool(name="wk", bufs=3) as wk, \
             tc.tile_pool(name="dram", bufs=1, space="DRAM") as dr:

            # ---- residents ----
            sb = {}
            for nm in wts:
                t = res.tile(list(wts[nm].shape), wts[nm].dtype, name=f"sb_{nm}")
                nc.sync.dma_start(out=t[:], in_=wts[nm][:])
                sb[nm] = t
            rr_sb = res.tile([P, WPC * cpt], _F32, name="rr_sb")
            nc.sync.dma_start(out=rr_sb[:], in_=rr_in[:, :])
            cc_sb = res.tile([P, WPC * cpt], _F32, name="cc_sb")
            nc.sync.dma_start(out=cc_sb[:], in_=cc_in_t[:, :])
            idx_sb = []
            for g in range(NG):
                t = res.tile([P, WPC * cpw_g[g] * 8], _I16, name=f"idx_sb{g}")
                nc.sync.dma_start(out=t[:], in_=idx_in[g][:, :])
                idx_sb.append(t)
            batch_sb = res.tile([P, WPC], _F32, name="batch_sb")
            nc.sync.dma_start(out=batch_sb[:], in_=batchf[:, :])
            hb_all = res.tile([P, WPC * H], _BF16, name="hb_all")
            znew_all = res.tile([P, WPC * H], _BF16, name="znew_all")

            # ---- DRAM state ----
            z_fulls = [dr.tile([NPAD, H], _BF16, addr_space="Shared", name=f"z_full{i}")
                       for i in range(NITER)]
            cc_in = dr.tile([P, WPC, H], _BF16, name="cc_in")
            ar_in = dr.tile([G, H], _F32, name="ar_in")
            ar_out = dr.tile([G, H], _F32, addr_space="Shared", name="ar_out")

            # ---- phase 1: encoder -> hb (=z1) ----
            with tc.tile_pool(name="psE", bufs=1, space="PSUM") as ps:
                for w in range(WPC):
                    xt = wk.tile([P, CIN], _F32, tag="xt")
                    nc.sync.dma_start(out=xt[:], in_=x_sh[:, w, :])
                    xTp = ps.tile([P, P], _F32, tag="tp")
                    nc.tensor.transpose(out=xTp[:], in_=xt[:], identity=sb["identf"][:])
                    xT = wk.tile([P, P], _BF16, tag="xT")
                    nc.scalar.activation(out=xT[:], in_=xTp[:], func=AF.Copy)
                    ps1 = ps.tile([64, P], _F32, tag="ps1")
                    nc.tensor.matmul(out=ps1[:], lhsT=sb["w1"][:], rhs=xT[:], start=True, stop=True)
                    l1 = wk.tile([64, P], _BF16, tag="l1")
                    nc.scalar.activation(out=l1[:], in_=ps1[:], func=AF.Relu, bias=sb["b1"][:, :1])
                    ps2 = ps.tile([H, P], _F32, tag="ps2")
                    nc.tensor.matmul(out=ps2[:], lhsT=sb["w2"][:], rhs=l1[:], start=True, stop=True)
                    l2 = wk.tile([H, P], _BF16, tag="l2")
                    nc.scalar.activation(out=l2[:], in_=ps2[:], func=AF.Relu, bias=sb["b2"][:, :1])
                    ps3 = ps.tile([H, P], _F32, tag="ps2b")
                    nc.tensor.matmul(out=ps3[:], lhsT=sb["w3"][:], rhs=l2[:], start=True, stop=True)
                    hbT = wk.tile([H, P], _BF16, tag="hbT")
                    nc.vector.tensor_scalar(out=hbT[:], in0=ps3[:],
                                            scalar1=sb["hb_scale"][:, :1], scalar2=sb["hb_bias"][:, :1],
                                            op0=OP.mult, op1=OP.add)
                    hbRp = ps.tile([P, P], _BF16, tag="tpb")
                    nc.tensor.transpose(out=hbRp[:], in_=hbT[:], identity=sb["ident"][:])
                    nc.scalar.activation(out=hb_all[:, w * H:(w + 1) * H], in_=hbRp[:], func=AF.Copy)
            nc.sync.dma_start(out=cc_in[:, :, :],
                              in_=hb_all[:].rearrange("p (w f) -> p w f", f=H))
            nc.gpsimd.collective_compute(
                "AllGather", OP.bypass, replica_groups=[list(range(NC))],
                ins=[cc_in.opt()], outs=[z_fulls[0].opt()])

            # ---- phase 2: fixed-point SpMM iterations ----
            nreg = [nc.gpsimd.to_reg(cpw_g[g] * P) for g in range(NG)]
            with tc.tile_pool(name="psS", bufs=2, space="PSUM") as ps:
                for t in range(niter):
                    last = (t == niter - 1)
                    z_full = z_fulls[t]
                    for w in range(WPC):
                        psw = ps.tile([P, H], _F32, tag="psw")
                        zgs = []
                        for g in range(NG):
                            K8 = cpw_g[g] * 8
                            zg = wk.tile([P, cpw_g[g] * H], _BF16, tag=f"zg{g}", name=f"zg{g}")
                            nc.gpsimd.dma_gather(
                                out_ap=zg[:].rearrange("p (a f) -> p a f", f=H),
                                in_ap=z_full[g * GSZ:(g + 1) * GSZ, :],
                                idxs_ap=idx_sb[g][:, w * K8:(w + 1) * K8],
                                num_idxs=cpw_g[g] * P,
                                num_idxs_reg=nreg[g],
                                elem_size=H,
                                single_packet=False,
                            )
                            zgs.append(zg)
                        first = True
                        for g in range(NG):
                            for k in range(cpw_g[g]):
                                colc = w * cpt + off_g[g] + k
                                st = wk.tile([P, P], _BF16, tag="st")
                                nc.vector.tensor_scalar(
                                    out=st[:], in0=sb["iota128"][:],
                                    scalar1=rr_sb[:, colc:colc + 1],
                                    scalar2=cc_sb[:, colc:colc + 1],
                                    op0=OP.is_equal, op1=OP.mult)
                                nc.tensor.matmul(
                                    out=psw[:], lhsT=st[:], rhs=zgs[g][:, k * H:(k + 1) * H],
                                    start=first, stop=(g == NG - 1 and k == cpw_g[g] - 1))
                                first = False
                        nc.vector.tensor_tensor(
                            out=znew_all[:, w * H:(w + 1) * H], in0=psw[:],
                            in1=hb_all[:, w * H:(w + 1) * H], op=OP.add)
                    if not last:
                        nc.sync.dma_start(out=cc_in[:, :, :],
                                          in_=znew_all[:].rearrange("p (w f) -> p w f", f=H))
                        nc.gpsimd.collective_compute(
                            "AllGather", OP.bypass, replica_groups=[list(range(NC))],
                            ins=[cc_in.opt()], outs=[z_fulls[t + 1].opt()])

            # ---- phase 3: node FC + pooling ----
            with tc.tile_pool(name="psQ", bufs=1, space="PSUM") as pq, \
                 tc.tile_pool(name="psF", bufs=2, space="PSUM") as ps:
                psq = [pq.tile([P, H], _F32, tag=f"poolq{q}", name=f"poolq{q}")
                       for q in range(4)]
                for w in range(WPC):
                    zTp = ps.tile([P, P], _BF16, tag="tp3")
                    nc.tensor.transpose(out=zTp[:], in_=znew_all[:, w * H:(w + 1) * H],
                                        identity=sb["ident"][:])
                    zT = wk.tile([P, P], _BF16, tag="zT3")
                    nc.scalar.activation(out=zT[:], in_=zTp[:], func=AF.Copy)
                    pf1 = ps.tile([H, P], _F32, tag="pf")
                    nc.tensor.matmul(out=pf1[:], lhsT=sb["fcw0"][:], rhs=zT[:], start=True, stop=True)
                    s1 = wk.tile([H, P], _BF16, tag="s1")
                    nc.scalar.activation(out=s1[:], in_=pf1[:], func=AF.Relu, bias=sb["fcb0"][:, :1])
                    pf2 = ps.tile([H, P], _F32, tag="pf")
                    nc.tensor.matmul(out=pf2[:], lhsT=sb["fcw1"][:], rhs=s1[:], start=True, stop=True)
                    s2T = wk.tile([H, P], _BF16, tag="s2T")
                    nc.scalar.activation(out=s2T[:], in_=pf2[:], func=AF.Relu, bias=sb["fcb1"][:, :1])
                    s2p = ps.tile([P, P], _BF16, tag="tp3")
                    nc.tensor.transpose(out=s2p[:], in_=s2T[:], identity=sb["ident"][:])
                    s2 = wk.tile([P, P], _BF16, tag="s2")
                    nc.scalar.activation(out=s2[:], in_=s2p[:], func=AF.Copy)
                    ind = wk.tile([P, G], _BF16, tag="ind")
                    nc.vector.tensor_scalar(out=ind[:], in0=sb["iota512"][:],
                                            scalar1=batch_sb[:, w:w + 1], scalar2=None,
                                            op0=OP.is_equal)
                    for q in range(4):
                        nc.tensor.matmul(out=psq[q][:], lhsT=ind[:, q * P:(q + 1) * P],
                                         rhs=s2[:], start=(w == 0), stop=(w == WPC - 1))
                pool_sb = wk.tile([P, 4 * H], _F32, tag="pool_sb", bufs=1)
                for q in range(4):
                    nc.vector.tensor_copy(out=pool_sb[:, q * H:(q + 1) * H], in_=psq[q][:])
                nc.sync.dma_start(out=ar_in[:, :].rearrange("(q p) f -> p q f", p=P),
                                  in_=pool_sb[:].rearrange("p (q f) -> p q f", f=H))
            nc.gpsimd.collective_compute(
                "AllReduce", OP.add, replica_groups=[list(range(NC))],
                ins=[ar_in.opt()], outs=[ar_out.opt()])

            # ---- phase 4: graph FC + log_softmax (redundant on all cores) ----
            with tc.tile_pool(name="psG", bufs=1, space="PSUM") as ps:
                gT = wk.tile([H, G], _BF16, tag="gT", bufs=1)
                for q in range(4):
                    gt = wk.tile([P, H], _F32, tag="gt")
                    nc.sync.dma_start(out=gt[:], in_=ar_out[q * P:(q + 1) * P, :])
                    gtp = ps.tile([P, P], _F32, tag="tp5")
                    nc.tensor.transpose(out=gtp[:], in_=gt[:], identity=sb["identf"][:])
                    nc.scalar.activation(out=gT[:, q * P:(q + 1) * P], in_=gtp[:], func=AF.Copy)
                pg1 = ps.tile([H, G], _F32, tag="pg")
                nc.tensor.matmul(out=pg1[:], lhsT=sb["gfcw0"][:], rhs=gT[:], start=True, stop=True)
                t1 = wk.tile([H, G], _BF16, tag="t1", bufs=1)
                nc.scalar.activation(out=t1[:], in_=pg1[:], func=AF.Relu, bias=sb["gfcb0"][:, :1])
                pg2 = ps.tile([H, G], _F32, tag="pg")
                nc.tensor.matmul(out=pg2[:], lhsT=sb["gfcw1"][:], rhs=t1[:], start=True, stop=True)
                t2 = wk.tile([H, G], _BF16, tag="t2", bufs=1)
                nc.scalar.activation(out=t2[:], in_=pg2[:], func=AF.Relu, bias=sb["gfcb1"][:, :1])
                pgf = ps.tile([P, G], _F32, tag="pg")
                nc.tensor.matmul(out=pgf[:COUT, :], lhsT=sb["finw"][:], rhs=t2[:], start=True, stop=True)
                f_sb = wk.tile([P, G], _F32, tag="f_sb", bufs=1)
                nc.gpsimd.memset(f_sb[:], 0.0)
                nc.vector.tensor_scalar(out=f_sb[:COUT, :], in0=pgf[:COUT, :],
                                        scalar1=sb["finb"][:COUT, :1], scalar2=None, op0=OP.add)
                for q in range(4):
                    ftp = ps.tile([P, P], _F32, tag="tp5")
                    nc.tensor.transpose(out=ftp[:], in_=f_sb[:, q * P:(q + 1) * P],
                                        identity=sb["identf"][:])
                    fr = wk.tile([P, P], _F32, tag="fr")
                    nc.vector.tensor_copy(out=fr[:], in_=ftp[:])
                    mx = wk.tile([P, 1], _F32, tag="mx")
                    nc.vector.tensor_reduce(out=mx[:], in_=fr[:, :COUT],
                                            axis=mybir.AxisListType.X, op=OP.max)
                    sh2 = wk.tile([P, COUT], _F32, tag="sh2")
                    nc.vector.tensor_scalar(out=sh2[:], in0=fr[:, :COUT], scalar1=mx[:, :1],
                                            scalar2=None, op0=OP.subtract)
                    ex = wk.tile([P, COUT], _F32, tag="ex")
                    nc.scalar.activation(out=ex[:], in_=sh2[:], func=AF.Exp)
                    sm = wk.tile([P, 1], _F32, tag="sm")
                    nc.vector.tensor_reduce(out=sm[:], in_=ex[:],
                                            axis=mybir.AxisListType.X, op=OP.add)
                    lg = wk.tile([P, 1], _F32, tag="lg")
                    nc.scalar.activation(out=lg[:], in_=sm[:], func=AF.Ln)
                    rs = wk.tile([P, COUT], _F32, tag="rs")
                    nc.vector.tensor_scalar(out=rs[:], in0=sh2[:], scalar1=lg[:, :1],
                                            scalar2=None, op0=OP.subtract)
                    nc.sync.dma_start(out=out[q * P:(q + 1) * P, :], in_=rs[:])
    nc.compile()
    return nc


_CACHE = {}


def kernel(**inputs):
    in_maps, cpw_g, cpt, off_g = _prep(inputs)
    key = tuple(cpw_g)
    if key not in _CACHE:
        _CACHE[key] = _build(cpw_g, cpt, off_g)
    nc = _CACHE[key]
    res = bass_utils.run_bass_kernel_spmd(nc, in_maps, core_ids=list(range(NC)))
    return np.asarray(res.results[0]["out"], np.float32)
